# revision 3
# baseline (speedup 1.0000x reference)
"""Trainium2 Bass kernel for nn_Block_86672440033530 (sparse_attention).

Transformer block: masked self-attention + AddNorm, class-vector cross-attn
(collapses to a broadcast row since Sk=1) + AddNorm, FFN + AddNorm.

Sharding: 8 cores = 2 batches x 4 query-blocks of 512 rows. Each core
computes full K/V projections for its batch (replicated within the 4-core
batch group), attention for its 512 query rows over all 2048 keys and all 16
heads, then output-projection / LayerNorms / FFN for its rows only. No
cross-core communication; host gathers the 8 row-blocks.

All matmuls run in bf16 (fp32 PSUM accumulation); norms/softmax plumbing in
fp32. The softmax is computed as exp(S)*mask with the denominator taken from
a ones-column appended to V (fused into the AV matmul) and divided out during
PSUM eviction via a DRAM-bounce partition broadcast.
"""
import contextlib
import ctypes
import sys
import types

import numpy as np

if "/opt/trn_rl_repo" not in sys.path:
    sys.path.insert(0, "/opt/trn_rl_repo")

import ml_dtypes  # noqa: E402
import concourse.bass as bass  # noqa: E402
import concourse.mybir as mybir  # noqa: E402
import concourse.tile as tile  # noqa: E402
from concourse.bass_utils import run_bass_kernel_spmd  # noqa: E402
from concourse.masks import make_identity  # noqa: E402

BF16 = mybir.dt.bfloat16
F32 = mybir.dt.float32
NP_BF16 = ml_dtypes.bfloat16

B, S, D, H, DFF = 2, 2048, 1024, 16, 4096
HD = D // H                      # 64
SCALE = float(1.0 / np.sqrt(np.float32(HD)))
NCORES = 8
QS = S // (NCORES // B)          # 512 query rows per core
QT = QS // 128                   # 4 query tiles per core
DT = D // 128                    # 8 d-blocks
ST = S // 128                    # 16 key tiles
FT = DFF // 128                  # 32 dff tiles
EPS = 1e-5


def _install_ntff_shim():
    """The axon image lacks antenv.axon_hooks; register the NTFF profile hook
    via ctypes so run_bass_kernel_spmd(trace=True) works. Harmless if unused."""
    try:
        import antenv
    except ImportError:
        return
    if "antenv.axon_hooks" in sys.modules:
        return

    def _make_hook(so_path):
        try:
            lib = ctypes.CDLL(so_path)
        except OSError:
            return None
        if not hasattr(lib, "axon_start_nrt_profile"):
            return None
        lib.axon_start_nrt_profile.argtypes = [
            ctypes.POINTER(ctypes.c_int64),
            ctypes.c_size_t,
        ]
        lib.axon_start_nrt_profile.restype = ctypes.c_int64
        lib.axon_stop_nrt_profile.argtypes = [ctypes.c_char_p]
        lib.axon_stop_nrt_profile.restype = ctypes.c_int64

        @contextlib.contextmanager
        def _hook(output_dir, device_ids):
            import jax

            jax.devices()
            if device_ids:
                ids = (ctypes.c_int64 * len(device_ids))(*device_ids)
                rc = lib.axon_start_nrt_profile(ids, len(device_ids))
            else:
                rc = lib.axon_start_nrt_profile(None, 0)
            if rc != 0:
                raise RuntimeError(f"axon_start_nrt_profile rc={rc}")
            try:
                yield
            finally:
                n = lib.axon_stop_nrt_profile(str(output_dir).encode())
                print(f"profile: {n} file(s) -> {output_dir}", file=sys.stderr)

        return _hook

    m = types.ModuleType("antenv.axon_hooks")
    m._hook = _make_hook("/opt/axon/libaxon_pjrt.so")
    m.set_axon_ntff_profile_hook = lambda h: setattr(m, "_hook", h)
    m.get_axon_ntff_profile_hook = lambda: m._hook
    sys.modules["antenv.axon_hooks"] = m
    import antenv

    antenv.axon_hooks = m


_install_ntff_shim()


def _split_sync_waits(nc, limit=1):
    """This walrus build accepts at most one sync-wait command per
    instruction; move excess waits onto same-engine NoOps placed before."""
    for func in nc.m.functions:
        for bb in func.blocks:
            out = []
            for ins in bb.instructions:
                si = getattr(ins, "sync_info", None)
                waits = list(si.on_wait) if (si is not None and si.on_wait) else []
                if len(waits) > limit:
                    keep, move = waits[:limit], waits[limit:]
                    for i in range(0, len(move), limit):
                        out.append(
                            mybir.InstNoOp(
                                name=f"{ins.name}-wsplit{i}",
                                sync_info=mybir.SyncInfo(
                                    on_wait=move[i : i + limit], on_update=[]
                                ),
                                bass_nofuse=True,
                                engine=ins.engine,
                            )
                        )
                    si.on_wait = keep
                out.append(ins)
            bb.instructions[:] = out


# ----------------------------------------------------------------------------
# device program (SPMD; identical on all 8 cores, per-core data differs)
# ----------------------------------------------------------------------------

def _build_program():
    nc = bass.Bass()

    def din(name, shape, dt):
        return nc.dram_tensor(name, list(shape), dt, kind="ExternalInput")

    # per-core tensors
    xT = din("xT", [128, DT, S], BF16)          # x[b].T  (d-major)
    xqT = din("xqT", [128, DT, QS], BF16)       # own q rows of xT
    xrows = din("xrows", [QS, D], F32)          # own q rows, natural (residual)
    maskT = din("maskT", [128, ST, QS], BF16)   # mask.T own q cols, tile-major
    cvT = din("cvT", [10, 1], BF16)             # classVector[b].T
    # weights (bf16, shared; wq/bq pre-scaled by 1/sqrt(hd))
    wq = din("wq", [D, D], BF16)
    wk = din("wk", [D, D], BF16)
    wv = din("wv", [D, D], BF16)
    wo = din("wo", [D, D], BF16)
    w1 = din("w1", [D, DFF], BF16)
    w2 = din("w2", [DFF, D], BF16)
    cew = din("cew", [10, D], BF16)
    cawv = din("cawv", [D, D], BF16)
    cawo = din("cawo", [D, D], BF16)
    # f32 bias/ln vectors: column-interleaved [128, n] or rows [1, n]
    bq_c = din("bq_c", [128, DT], F32)
    bk_c = din("bk_c", [128, DT], F32)
    b1_c = din("b1_c", [128, FT], F32)
    ceb_c = din("ceb_c", [128, DT], F32)
    cabv_c = din("cabv_c", [128, DT], F32)
    cabo_c = din("cabo_c", [128, DT], F32)
    bv_r = din("bv_r", [1, D], F32)
    bo_r = din("bo_r", [1, D], F32)
    b2_r = din("b2_r", [1, D], F32)
    g1_r = din("g1_r", [1, D], F32)
    lb1_r = din("lb1_r", [1, D], F32)
    g2_r = din("g2_r", [1, D], F32)
    lb2_r = din("lb2_r", [1, D], F32)
    g3_r = din("g3_r", [1, D], F32)
    lb3_r = din("lb3_r", [1, D], F32)

    out_d = nc.dram_tensor("out", [QS, D], F32, kind="ExternalOutput")

    Exp = mybir.ActivationFunctionType.Exp
    Relu = mybir.ActivationFunctionType.Relu
    Sqrt = mybir.ActivationFunctionType.Sqrt
    ADD = mybir.AluOpType.add
    SUB = mybir.AluOpType.subtract
    MUL = mybir.AluOpType.mult

    with tile.TileContext(nc) as tc, contextlib.ExitStack() as ctx:
        # -------- whole-kernel residents (small) ---------------------------
        res = ctx.enter_context(tc.tile_pool(name="res", bufs=1))
        dres = ctx.enter_context(tc.tile_pool(name="dres", bufs=1, space="DRAM"))

        ident = res.tile([128, 128], BF16)
        make_identity(nc, ident)
        eps_t = res.tile([128, 1], F32)
        nc.vector.memset(eps_t[:], EPS)
        oT_s = res.tile([128, DT, QS], BF16)     # attention output (transposed)
        r_b = res.tile([128, D], F32)            # cross-attn row, broadcast

        def bcast_load(pool, src_row, n, tag):
            t = pool.tile([128, n], F32, tag=tag)
            nc.sync.dma_start(out=t[:], in_=src_row[0:1, :].broadcast_to((128, n)))
            return t

        def layer_norm(pool, dst, src, g_b, lb_b):
            """dst = LN_freedim(src) * g + b for [128, D] f32 views."""
            stats = pool.tile([128, 2, 6], F32, tag="lnst")
            mv = pool.tile([128, 2], F32, tag="lnmv")
            for sg in range(2):
                nc.vector.bn_stats(
                    out=stats[:, sg, :], in_=src[:, sg * 512 : (sg + 1) * 512]
                )
            nc.vector.bn_aggr(out=mv[:], in_=stats[:])
            rstd = pool.tile([128, 1], F32, tag="lnrs")
            nc.scalar.activation(
                out=rstd[:], in_=mv[:, 1:2], func=Sqrt, bias=eps_t[:]
            )
            nc.vector.reciprocal(out=rstd[:], in_=rstd[:])
            nc.vector.tensor_scalar(
                out=dst[:], in0=src[:], scalar1=mv[:, 0:1], scalar2=rstd[:],
                op0=SUB, op1=MUL,
            )
            nc.vector.tensor_mul(out=dst[:], in0=dst[:], in1=g_b[:])
            nc.vector.tensor_add(out=dst[:], in0=dst[:], in1=lb_b[:])

        # -------- phase R: cross-attn row r --------------------------------
        # r = ((classVector @ ce_w + ce_b) @ ca_wv + ca_bv) @ ca_wo + ca_bo,
        # in column form rT [128, DT]; softmax over a single key == 1 so the
        # q/k cross-attn projections drop out entirely.
        with tc.tile_pool(name="rph", bufs=1) as rp, \
             tc.tile_pool(name="rps", bufs=2, space="PSUM") as rps:
            cv_bf = rp.tile([10, 1], BF16)
            nc.sync.dma_start(out=cv_bf[:], in_=cvT[:])
            cew_s = rp.tile([10, D], BF16)
            nc.sync.dma_start(out=cew_s[:], in_=cew[:])
            ceb_s = rp.tile([128, DT], F32)
            nc.sync.dma_start(out=ceb_s[:], in_=ceb_c[:])
            cabv_s = rp.tile([128, DT], F32)
            nc.sync.dma_start(out=cabv_s[:], in_=cabv_c[:])
            cabo_s = rp.tile([128, DT], F32)
            nc.sync.dma_start(out=cabo_s[:], in_=cabo_c[:])

            cv_ps = rps.tile([128, DT], F32, tag="rp1")
            for m in range(DT):
                nc.tensor.matmul(
                    cv_ps[:, m : m + 1], cew_s[:, m * 128 : (m + 1) * 128],
                    cv_bf[:], start=True, stop=True,
                )
            cvec = rp.tile([128, DT], BF16)
            nc.vector.tensor_add(out=cvec[:], in0=cv_ps[:], in1=ceb_s[:])

            vcv_ps = rps.tile([128, DT], F32, tag="rp2")
            cawv_blk = rp.tile([128, DT, D], BF16, tag="rwblk")
            nc.sync.dma_start(
                out=cawv_blk[:], in_=cawv.rearrange("(a p) n -> p a n", p=128)
            )
            for m in range(DT):
                for k in range(DT):
                    nc.tensor.matmul(
                        vcv_ps[:, m : m + 1],
                        cawv_blk[:, k, m * 128 : (m + 1) * 128],
                        cvec[:, k : k + 1],
                        start=(k == 0), stop=(k == DT - 1),
                    )
            vcv = rp.tile([128, DT], BF16)
            nc.vector.tensor_add(out=vcv[:], in0=vcv_ps[:], in1=cabv_s[:])

            r_ps = rps.tile([128, DT], F32, tag="rp3")
            cawo_blk = rp.tile([128, DT, D], BF16, tag="rwblk2")
            nc.sync.dma_start(
                out=cawo_blk[:], in_=cawo.rearrange("(a p) n -> p a n", p=128)
            )
            for m in range(DT):
                for k in range(DT):
                    nc.tensor.matmul(
                        r_ps[:, m : m + 1],
                        cawo_blk[:, k, m * 128 : (m + 1) * 128],
                        vcv[:, k : k + 1],
                        start=(k == 0), stop=(k == DT - 1),
                    )
            rT = rp.tile([128, DT], F32)
            nc.vector.tensor_add(out=rT[:], in0=r_ps[:], in1=cabo_s[:])
            r_dram = dres.tile([D], F32)
            nc.sync.dma_start(
                out=r_dram.rearrange("(a p) -> p a", p=128), in_=rT[:]
            )
            nc.sync.dma_start(
                out=r_b[:], in_=r_dram[None, :].broadcast_to((128, D))
            )

        # -------- phases P+A share the big attention residents -------------
        with tc.tile_pool(name="pa", bufs=1) as pa:
            kT_s = pa.tile([128, DT, S], BF16)          # K.T (d-major), +bk
            vp_s = pa.tile([128, ST, H, HD + 1], BF16)  # V natural + ones col
            qT_s = pa.tile([128, DT, QS], BF16)         # Q.T (pre-scaled)
            maskT_s = pa.tile([128, ST, QS], BF16)
            nc.sync.dma_start(out=maskT_s[:], in_=maskT[:])
            xqT_s = pa.tile([128, DT, QS], BF16)
            nc.sync.dma_start(out=xqT_s[:], in_=xqT[:])

            # ---- phase P: K/V/Q projections -------------------------------
            with tc.tile_pool(name="pph", bufs=1) as pp, \
                 tc.tile_pool(name="pw", bufs=12) as pw, \
                 tc.tile_pool(name="pps", bufs=4, space="PSUM") as pps:
                xT_s = pp.tile([128, DT, S], BF16)
                nc.sync.dma_start(out=xT_s[:], in_=xT[:])
                bq_s = pp.tile([128, DT], F32)
                nc.sync.dma_start(out=bq_s[:], in_=bq_c[:])
                bk_s = pp.tile([128, DT], F32)
                nc.sync.dma_start(out=bk_s[:], in_=bk_c[:])
                bv_b = bcast_load(pp, bv_r, D, "bvb")

                nc.vector.memset(vp_s[:, :, :, HD : HD + 1], 1.0)

                # K: kT[:, m, ns] = wk[:, m].T @ xT  (+bk)
                wk_t = [pw.tile([128, D], BF16, tag="w", name=f"wk_t{_k}") for _k in range(DT)]
                for k in range(DT):
                    nc.sync.dma_start(
                        out=wk_t[k][:], in_=wk[k * 128 : (k + 1) * 128, :]
                    )
                for m in range(DT):
                    for ns in range(S // 512):
                        ps = pps.tile([128, 512], F32, tag="pj")
                        for k in range(DT):
                            nc.tensor.matmul(
                                ps[:],
                                wk_t[k][:, m * 128 : (m + 1) * 128],
                                xT_s[:, k, ns * 512 : (ns + 1) * 512],
                                start=(k == 0), stop=(k == DT - 1),
                            )
                        nc.vector.tensor_scalar(
                            out=kT_s[:, m, ns * 512 : (ns + 1) * 512],
                            in0=ps[:], scalar1=bk_s[:, m : m + 1], scalar2=None,
                            op0=ADD,
                        )

                # Q (own rows): qT[:, m, :] = wq[:, m].T @ xqT  (+bq)
                wq_t = [pw.tile([128, D], BF16, tag="w", name=f"wq_t{_k}") for _k in range(DT)]
                for k in range(DT):
                    nc.sync.dma_start(
                        out=wq_t[k][:], in_=wq[k * 128 : (k + 1) * 128, :]
                    )
                for m in range(DT):
                    ps = pps.tile([128, QS], F32, tag="pj")
                    for k in range(DT):
                        nc.tensor.matmul(
                            ps[:],
                            wq_t[k][:, m * 128 : (m + 1) * 128],
                            xqT_s[:, k, :],
                            start=(k == 0), stop=(k == DT - 1),
                        )
                    nc.vector.tensor_scalar(
                        out=qT_s[:, m, :], in0=ps[:],
                        scalar1=bq_s[:, m : m + 1], scalar2=None, op0=ADD,
                    )

                # V (natural): vp[:, st, heads, :64] = xT[:, :, st].T @ wv (+bv)
                wv_t = [pw.tile([128, D], BF16, tag="w", name=f"wv_t{_k}") for _k in range(DT)]
                for k in range(DT):
                    nc.sync.dma_start(
                        out=wv_t[k][:], in_=wv[k * 128 : (k + 1) * 128, :]
                    )
                for st in range(ST):
                    for c in range(D // 512):
                        ps = pps.tile([128, 512], F32, tag="pj")
                        for k in range(DT):
                            nc.tensor.matmul(
                                ps[:],
                                xT_s[:, k, st * 128 : (st + 1) * 128],
                                wv_t[k][:, c * 512 : (c + 1) * 512],
                                start=(k == 0), stop=(k == DT - 1),
                            )
                        nc.vector.tensor_add(
                            out=vp_s[:, st, c * 8 : (c + 1) * 8, 0:HD],
                            in0=ps[:].rearrange("p (h e) -> p h e", e=HD),
                            in1=bv_b[:, c * 512 : (c + 1) * 512].rearrange(
                                "p (h e) -> p h e", e=HD
                            ),
                        )

            # ---- phase A: attention ---------------------------------------
            with tc.tile_pool(name="aph", bufs=3) as apl, \
                 tc.tile_pool(name="aps", bufs=2, space="PSUM") as aps, \
                 tc.tile_pool(name="avps", bufs=2, space="PSUM") as avps, \
                 tc.tile_pool(name="adr", bufs=3, space="DRAM") as adr:
                for h in range(H):
                    pb = (h % 2) * 64
                    dtile = h // 2
                    av = avps.tile([HD + 1, QS], F32, tag="av")
                    for jp in range(ST // 2):
                        qk = aps.tile([128, 2, 512], F32, tag="qk")
                        for hf in range(2):
                            j = jp * 2 + hf
                            nc.tensor.matmul(
                                qk[:, hf, :],
                                kT_s[pb : pb + 64, dtile, j * 128 : (j + 1) * 128],
                                qT_s[pb : pb + 64, dtile, :],
                                start=True, stop=True,
                            )
                        pt = apl.tile([128, 2, 512], BF16, tag="pt")
                        nc.scalar.activation(pt[:], qk[:], Exp)
                        nc.vector.tensor_mul(
                            out=pt[:], in0=pt[:],
                            in1=maskT_s[:, jp * 2 : jp * 2 + 2, :],
                        )
                        for hf in range(2):
                            j = jp * 2 + hf
                            nc.tensor.matmul(
                                av[:],
                                vp_s[:, j, h, :],
                                pt[:, hf, :],
                                start=(j == 0), stop=(j == ST - 1),
                            )
                    # divide by the ones-column sum, store transposed bf16
                    dr_row = apl.tile([1, QS], F32, tag="dr")
                    nc.vector.reciprocal(out=dr_row[:], in_=av[HD : HD + 1, :])
                    dnb = adr.tile([1, QS], F32, tag="dnb")
                    nc.sync.dma_start(out=dnb[:], in_=dr_row[:])
                    rb = apl.tile([64, QS], F32, tag="rb")
                    nc.sync.dma_start(
                        out=rb[:], in_=dnb[0:1, :].broadcast_to((64, QS))
                    )
                    nc.vector.tensor_mul(
                        out=oT_s[pb : pb + 64, dtile, :], in0=av[0:HD, :],
                        in1=rb[:],
                    )

        # -------- phase O: out-proj, AddNorm, cross-attn row, AddNorm ------
        with tc.tile_pool(name="of", bufs=1) as of:
            h2_s = of.tile([128, QT, D], F32)
            h2T_s = of.tile([128, DT, QS], BF16)
            with tc.tile_pool(name="oph", bufs=1) as op, \
                 tc.tile_pool(name="ow", bufs=9) as ow, \
                 tc.tile_pool(name="ops", bufs=4, space="PSUM") as ops, \
                 tc.tile_pool(name="otps", bufs=2, space="PSUM") as otps, \
                 tc.tile_pool(name="oln", bufs=4) as oln:
                xr_s = op.tile([128, QT, D], F32)
                nc.sync.dma_start(
                    out=xr_s[:], in_=xrows.rearrange("(t p) d -> p t d", p=128)
                )
                bo_b = bcast_load(op, bo_r, D, "bob")
                g1_b = bcast_load(op, g1_r, D, "g1b")
                lb1_b = bcast_load(op, lb1_r, D, "lb1b")
                g2_b = bcast_load(op, g2_r, D, "g2b")
                lb2_b = bcast_load(op, lb2_r, D, "lb2b")

                wo_t = [ow.tile([128, D], BF16, tag="wo", name=f"wo_t{_k}") for _k in range(DT)]
                for k in range(DT):
                    nc.sync.dma_start(
                        out=wo_t[k][:], in_=wo[k * 128 : (k + 1) * 128, :]
                    )

                h_s = op.tile([128, QT, D], F32)
                for mq in range(QT):
                    for ns in range(D // 512):
                        ps = ops.tile([128, 512], F32, tag="op")
                        for k in range(DT):
                            nc.tensor.matmul(
                                ps[:],
                                oT_s[:, k, mq * 128 : (mq + 1) * 128],
                                wo_t[k][:, ns * 512 : (ns + 1) * 512],
                                start=(k == 0), stop=(k == DT - 1),
                            )
                        sl = slice(ns * 512, (ns + 1) * 512)
                        nc.vector.tensor_add(
                            out=h_s[:, mq, sl], in0=ps[:], in1=bo_b[:, sl]
                        )
                        nc.vector.tensor_add(
                            out=h_s[:, mq, sl], in0=h_s[:, mq, sl],
                            in1=xr_s[:, mq, sl],
                        )
                    layer_norm(oln, h_s[:, mq, :], h_s[:, mq, :], g1_b, lb1_b)
                    nc.vector.tensor_add(
                        out=h2_s[:, mq, :], in0=h_s[:, mq, :], in1=r_b[:]
                    )
                    layer_norm(oln, h2_s[:, mq, :], h2_s[:, mq, :], g2_b, lb2_b)
                    h2bf = oln.tile([128, D], BF16, tag="h2bf")
                    nc.vector.tensor_copy(out=h2bf[:], in_=h2_s[:, mq, :])
                    for t in range(DT):
                        tp = otps.tile([128, 128], BF16, tag="tp")
                        nc.tensor.transpose(
                            tp[:], h2bf[:, t * 128 : (t + 1) * 128], ident[:]
                        )
                        nc.vector.tensor_copy(
                            out=h2T_s[:, t, mq * 128 : (mq + 1) * 128], in_=tp[:]
                        )

            # -------- phase F: FFN + AddNorm -------------------------------
            with tc.tile_pool(name="fph", bufs=1) as fp, \
                 tc.tile_pool(name="fln", bufs=4) as fln:
                b1_s = fp.tile([128, FT], F32)
                nc.sync.dma_start(out=b1_s[:], in_=b1_c[:])
                b2_b = bcast_load(fp, b2_r, D, "b2b")
                g3_b = bcast_load(fp, g3_r, D, "g3b")
                lb3_b = bcast_load(fp, lb3_r, D, "lb3b")

                fT_s = fp.tile([128, FT, QS], BF16)
                # FF1: fT[:, mf, :] = relu(w1[:, mf].T @ h2T + b1)
                with tc.tile_pool(name="fw1", bufs=2) as fw1, \
                     tc.tile_pool(name="fps", bufs=3, space="PSUM") as fps:
                    for mfg in range(4):
                        w1_t = fw1.tile([128, DT, 1024], BF16, tag="w1")
                        nc.sync.dma_start(
                            out=w1_t[:],
                            in_=w1.rearrange("(a p) n -> p a n", p=128)[
                                :, :, mfg * 1024 : (mfg + 1) * 1024
                            ],
                        )
                        for mfl in range(8):
                            mf = mfg * 8 + mfl
                            ps = fps.tile([128, QS], F32, tag="f1")
                            for k in range(DT):
                                nc.tensor.matmul(
                                    ps[:],
                                    w1_t[:, k, mfl * 128 : (mfl + 1) * 128],
                                    h2T_s[:, k, :],
                                    start=(k == 0), stop=(k == DT - 1),
                                )
                            nc.vector.tensor_scalar(
                                out=ps[:], in0=ps[:],
                                scalar1=b1_s[:, mf : mf + 1], scalar2=None,
                                op0=ADD,
                            )
                            nc.scalar.activation(
                                out=fT_s[:, mf, :], in_=ps[:], func=Relu
                            )

                # FF2: ff rows = fT.T @ w2 (+b2) + h2, then LN3
                out_t = fp.tile([128, QT, D], F32)
                with tc.tile_pool(name="fw2", bufs=3) as fw2, \
                     tc.tile_pool(name="f2ps", bufs=8, space="PSUM") as f2ps:
                    ps2 = [
                        f2ps.tile([128, 512], F32, tag="f2", name=f"ps2_{_i}")
                        for _i in range(QT * (D // 512))
                    ]
                    for kf in range(FT):
                        w2_t = fw2.tile([128, D], BF16, tag="w2")
                        nc.sync.dma_start(
                            out=w2_t[:], in_=w2[kf * 128 : (kf + 1) * 128, :]
                        )
                        for mq in range(QT):
                            for ns in range(D // 512):
                                nc.tensor.matmul(
                                    ps2[mq * 2 + ns][:],
                                    fT_s[:, kf, mq * 128 : (mq + 1) * 128],
                                    w2_t[:, ns * 512 : (ns + 1) * 512],
                                    start=(kf == 0), stop=(kf == FT - 1),
                                )
                    for mq in range(QT):
                        for ns in range(D // 512):
                            sl = slice(ns * 512, (ns + 1) * 512)
                            nc.vector.tensor_add(
                                out=out_t[:, mq, sl], in0=ps2[mq * 2 + ns][:],
                                in1=b2_b[:, sl],
                            )
                            nc.vector.tensor_add(
                                out=out_t[:, mq, sl], in0=out_t[:, mq, sl],
                                in1=h2_s[:, mq, sl],
                            )
                        layer_norm(
                            fln, out_t[:, mq, :], out_t[:, mq, :], g3_b, lb3_b
                        )
                        nc.sync.dma_start(
                            out=out_d.rearrange("(t p) d -> p t d", p=128)[:, mq, :],
                            in_=out_t[:, mq, :],
                        )

    _split_sync_waits(nc)
    return nc


_NC_CACHE = None


def _get_program():
    global _NC_CACHE
    if _NC_CACHE is None:
        _NC_CACHE = _build_program()
    return _NC_CACHE


# ----------------------------------------------------------------------------
# host wrapper
# ----------------------------------------------------------------------------

def _col_interleave(v, nt):
    """[n] f32 -> [128, nt] where col j holds v[j*128:(j+1)*128]."""
    return np.ascontiguousarray(
        np.asarray(v, np.float32).reshape(nt, 128).T
    )


def kernel(**inputs):
    x = np.asarray(inputs["cur_input"], np.float32)          # [B, S, D]
    cls = np.asarray(inputs["classVector"], np.float32)      # [B, 1, 10]
    mask = np.asarray(inputs["attn_mask"])                   # [S, S] bool

    bf = lambda a: np.ascontiguousarray(np.asarray(a, np.float32)).astype(NP_BF16)
    f32 = lambda a: np.ascontiguousarray(np.asarray(a, np.float32))
    row = lambda v: f32(np.asarray(v, np.float32).reshape(1, -1))

    shared = dict(
        wq=bf(np.asarray(inputs["sa_wq"], np.float32) * SCALE),
        wk=bf(inputs["sa_wk"]),
        wv=bf(inputs["sa_wv"]),
        wo=bf(inputs["sa_wo"]),
        w1=bf(inputs["ff_w1"]),
        w2=bf(inputs["ff_w2"]),
        cew=bf(inputs["ce_w"]),
        cawv=bf(inputs["ca_wv"]),
        cawo=bf(inputs["ca_wo"]),
        bq_c=_col_interleave(np.asarray(inputs["sa_bq"], np.float32) * SCALE, DT),
        bk_c=_col_interleave(inputs["sa_bk"], DT),
        b1_c=_col_interleave(inputs["ff_b1"], FT),
        ceb_c=_col_interleave(inputs["ce_b"], DT),
        cabv_c=_col_interleave(inputs["ca_bv"], DT),
        cabo_c=_col_interleave(inputs["ca_bo"], DT),
        bv_r=row(inputs["sa_bv"]),
        bo_r=row(inputs["sa_bo"]),
        b2_r=row(inputs["ff_b2"]),
        g1_r=row(inputs["ln1_g"]),
        lb1_r=row(inputs["ln1_b"]),
        g2_r=row(inputs["ln2_g"]),
        lb2_r=row(inputs["ln2_b"]),
        g3_r=row(inputs["ln3_g"]),
        lb3_r=row(inputs["ln3_b"]),
    )

    mT = mask.T.astype(np.float32)  # [S key, S query]
    in_maps = []
    for c in range(NCORES):
        b, q0 = c // (NCORES // B), (c % (NCORES // B)) * QS
        xTb = x[b].T.reshape(DT, 128, S).transpose(1, 0, 2)       # [128, DT, S]
        mTc = mT[:, q0 : q0 + QS].reshape(ST, 128, QS).transpose(1, 0, 2)
        in_maps.append(
            dict(
                shared,
                xT=bf(xTb),
                xqT=bf(xTb[:, :, q0 : q0 + QS]),
                xrows=f32(x[b, q0 : q0 + QS, :]),
                maskT=bf(mTc),
                cvT=bf(cls[b, 0].reshape(10, 1)),
            )
        )

    res = run_bass_kernel_spmd(_get_program(), in_maps, list(range(NCORES)))
    out = np.empty((B, S, D), np.float32)
    for c in range(NCORES):
        b, q0 = c // (NCORES // B), (c % (NCORES // B)) * QS
        out[b, q0 : q0 + QS] = res.results[c]["out"]
    return out


# revision 4
# speedup vs baseline: 1.0508x; 1.0508x over previous
"""Trainium2 Bass kernel for nn_Block_86672440033530 (sparse_attention).

Transformer block: masked self-attention + AddNorm, class-vector cross-attn
(collapses to a broadcast row since Sk=1) + AddNorm, FFN + AddNorm.

Sharding: 8 cores = 2 batches x 4 query-blocks of 512 rows. Each core
computes full K/V projections for its batch (replicated within the 4-core
batch group), attention for its 512 query rows over all 2048 keys and all 16
heads, then output-projection / LayerNorms / FFN for its rows only. No
cross-core communication; host gathers the 8 row-blocks.

All matmuls run in bf16 (fp32 PSUM accumulation); norms/softmax plumbing in
fp32. The softmax is computed as exp(S)*mask with the denominator taken from
a ones-column appended to V (fused into the AV matmul) and divided out during
PSUM eviction via a DRAM-bounce partition broadcast.
"""
import contextlib
import ctypes
import sys
import types

import numpy as np

if "/opt/trn_rl_repo" not in sys.path:
    sys.path.insert(0, "/opt/trn_rl_repo")

import ml_dtypes  # noqa: E402
import concourse.bass as bass  # noqa: E402
import concourse.mybir as mybir  # noqa: E402
import concourse.tile as tile  # noqa: E402
from concourse.bass_utils import run_bass_kernel_spmd  # noqa: E402
from concourse.masks import make_identity  # noqa: E402

BF16 = mybir.dt.bfloat16
F32 = mybir.dt.float32
NP_BF16 = ml_dtypes.bfloat16

B, S, D, H, DFF = 2, 2048, 1024, 16, 4096
HD = D // H                      # 64
SCALE = float(1.0 / np.sqrt(np.float32(HD)))
NCORES = 8
QS = S // (NCORES // B)          # 512 query rows per core
QT = QS // 128                   # 4 query tiles per core
DT = D // 128                    # 8 d-blocks
ST = S // 128                    # 16 key tiles
FT = DFF // 128                  # 32 dff tiles
EPS = 1e-5


def _install_ntff_shim():
    """The axon image lacks antenv.axon_hooks; register the NTFF profile hook
    via ctypes so run_bass_kernel_spmd(trace=True) works. Harmless if unused."""
    try:
        import antenv
    except ImportError:
        return
    if "antenv.axon_hooks" in sys.modules:
        return

    def _make_hook(so_path):
        try:
            lib = ctypes.CDLL(so_path)
        except OSError:
            return None
        if not hasattr(lib, "axon_start_nrt_profile"):
            return None
        lib.axon_start_nrt_profile.argtypes = [
            ctypes.POINTER(ctypes.c_int64),
            ctypes.c_size_t,
        ]
        lib.axon_start_nrt_profile.restype = ctypes.c_int64
        lib.axon_stop_nrt_profile.argtypes = [ctypes.c_char_p]
        lib.axon_stop_nrt_profile.restype = ctypes.c_int64

        @contextlib.contextmanager
        def _hook(output_dir, device_ids):
            import jax

            jax.devices()
            if device_ids:
                ids = (ctypes.c_int64 * len(device_ids))(*device_ids)
                rc = lib.axon_start_nrt_profile(ids, len(device_ids))
            else:
                rc = lib.axon_start_nrt_profile(None, 0)
            if rc != 0:
                raise RuntimeError(f"axon_start_nrt_profile rc={rc}")
            try:
                yield
            finally:
                n = lib.axon_stop_nrt_profile(str(output_dir).encode())
                print(f"profile: {n} file(s) -> {output_dir}", file=sys.stderr)

        return _hook

    m = types.ModuleType("antenv.axon_hooks")
    m._hook = _make_hook("/opt/axon/libaxon_pjrt.so")
    m.set_axon_ntff_profile_hook = lambda h: setattr(m, "_hook", h)
    m.get_axon_ntff_profile_hook = lambda: m._hook
    sys.modules["antenv.axon_hooks"] = m
    import antenv

    antenv.axon_hooks = m


_install_ntff_shim()


def _split_sync_waits(nc, limit=1):
    """This walrus build accepts at most one sync-wait command per
    instruction; move excess waits onto same-engine NoOps placed before."""
    for func in nc.m.functions:
        for bb in func.blocks:
            out = []
            for ins in bb.instructions:
                si = getattr(ins, "sync_info", None)
                waits = list(si.on_wait) if (si is not None and si.on_wait) else []
                if len(waits) > limit:
                    keep, move = waits[:limit], waits[limit:]
                    for i in range(0, len(move), limit):
                        out.append(
                            mybir.InstNoOp(
                                name=f"{ins.name}-wsplit{i}",
                                sync_info=mybir.SyncInfo(
                                    on_wait=move[i : i + limit], on_update=[]
                                ),
                                bass_nofuse=True,
                                engine=ins.engine,
                            )
                        )
                    si.on_wait = keep
                out.append(ins)
            bb.instructions[:] = out


# ----------------------------------------------------------------------------
# device program (SPMD; identical on all 8 cores, per-core data differs)
# ----------------------------------------------------------------------------

def _build_program():
    nc = bass.Bass()

    def din(name, shape, dt):
        return nc.dram_tensor(name, list(shape), dt, kind="ExternalInput")

    # per-core tensors
    xT = din("xT", [128, DT, S], BF16)          # x[b].T  (d-major)
    xqT = din("xqT", [128, DT, QS], BF16)       # own q rows of xT
    xrows = din("xrows", [QS, D], F32)          # own q rows, natural (residual)
    maskT = din("maskT", [128, ST, QS], BF16)   # mask.T own q cols, tile-major
    cvT = din("cvT", [10, 1], BF16)             # classVector[b].T
    # weights (bf16, shared; wq/bq pre-scaled by 1/sqrt(hd))
    wq = din("wq", [D, D], BF16)
    wk = din("wk", [D, D], BF16)
    wv = din("wv", [D, D], BF16)
    wo = din("wo", [D, D], BF16)
    w1 = din("w1", [D, DFF], BF16)
    w2 = din("w2", [DFF, D], BF16)
    cew = din("cew", [10, D], BF16)
    cawv = din("cawv", [D, D], BF16)
    cawo = din("cawo", [D, D], BF16)
    # f32 bias/ln vectors: column-interleaved [128, n] or rows [1, n]
    bq_c = din("bq_c", [128, DT], F32)
    bk_c = din("bk_c", [128, DT], F32)
    b1_c = din("b1_c", [128, FT], F32)
    ceb_c = din("ceb_c", [128, DT], F32)
    cabv_c = din("cabv_c", [128, DT], F32)
    cabo_c = din("cabo_c", [128, DT], F32)
    bv_r = din("bv_r", [1, D], F32)
    bo_r = din("bo_r", [1, D], F32)
    b2_r = din("b2_r", [1, D], F32)
    g1_r = din("g1_r", [1, D], F32)
    lb1_r = din("lb1_r", [1, D], F32)
    g2_r = din("g2_r", [1, D], F32)
    lb2_r = din("lb2_r", [1, D], F32)
    g3_r = din("g3_r", [1, D], F32)
    lb3_r = din("lb3_r", [1, D], F32)

    out_d = nc.dram_tensor("out", [QS, D], F32, kind="ExternalOutput")

    Exp = mybir.ActivationFunctionType.Exp
    Relu = mybir.ActivationFunctionType.Relu
    Sqrt = mybir.ActivationFunctionType.Sqrt
    ADD = mybir.AluOpType.add
    SUB = mybir.AluOpType.subtract
    MUL = mybir.AluOpType.mult

    with tile.TileContext(nc) as tc, contextlib.ExitStack() as ctx:
        # -------- whole-kernel residents (small) ---------------------------
        res = ctx.enter_context(tc.tile_pool(name="res", bufs=1))
        dres = ctx.enter_context(tc.tile_pool(name="dres", bufs=1, space="DRAM"))

        ident = res.tile([128, 128], BF16)
        make_identity(nc, ident)
        eps_t = res.tile([128, 1], F32)
        nc.vector.memset(eps_t[:], EPS)
        oT_s = res.tile([128, DT, QS], BF16)     # attention output (transposed)
        r_b = res.tile([128, D], F32)            # cross-attn row, broadcast

        def bcast_load(pool, src_row, n, tag):
            t = pool.tile([128, n], F32, tag=tag)
            nc.sync.dma_start(out=t[:], in_=src_row[0:1, :].broadcast_to((128, n)))
            return t

        def layer_norm(pool, dst, src, g_b, lb_b):
            """dst = LN_freedim(src) * g + b for [128, D] f32 views."""
            stats = pool.tile([128, 2, 6], F32, tag="lnst")
            mv = pool.tile([128, 2], F32, tag="lnmv")
            for sg in range(2):
                nc.vector.bn_stats(
                    out=stats[:, sg, :], in_=src[:, sg * 512 : (sg + 1) * 512]
                )
            nc.vector.bn_aggr(out=mv[:], in_=stats[:])
            rstd = pool.tile([128, 1], F32, tag="lnrs")
            nc.scalar.activation(
                out=rstd[:], in_=mv[:, 1:2], func=Sqrt, bias=eps_t[:]
            )
            nc.vector.reciprocal(out=rstd[:], in_=rstd[:])
            nc.vector.tensor_scalar(
                out=dst[:], in0=src[:], scalar1=mv[:, 0:1], scalar2=rstd[:],
                op0=SUB, op1=MUL,
            )
            nc.vector.tensor_mul(out=dst[:], in0=dst[:], in1=g_b[:])
            nc.vector.tensor_add(out=dst[:], in0=dst[:], in1=lb_b[:])

        # -------- phase R: cross-attn row r --------------------------------
        # r = ((classVector @ ce_w + ce_b) @ ca_wv + ca_bv) @ ca_wo + ca_bo,
        # in column form rT [128, DT]; softmax over a single key == 1 so the
        # q/k cross-attn projections drop out entirely.
        with tc.tile_pool(name="rph", bufs=1) as rp, \
             tc.tile_pool(name="rps", bufs=2, space="PSUM") as rps:
            cv_bf = rp.tile([10, 1], BF16)
            nc.sync.dma_start(out=cv_bf[:], in_=cvT[:])
            cew_s = rp.tile([10, D], BF16)
            nc.sync.dma_start(out=cew_s[:], in_=cew[:])
            ceb_s = rp.tile([128, DT], F32)
            nc.sync.dma_start(out=ceb_s[:], in_=ceb_c[:])
            cabv_s = rp.tile([128, DT], F32)
            nc.sync.dma_start(out=cabv_s[:], in_=cabv_c[:])
            cabo_s = rp.tile([128, DT], F32)
            nc.sync.dma_start(out=cabo_s[:], in_=cabo_c[:])

            cv_ps = rps.tile([128, DT], F32, tag="rp1")
            for m in range(DT):
                nc.tensor.matmul(
                    cv_ps[:, m : m + 1], cew_s[:, m * 128 : (m + 1) * 128],
                    cv_bf[:], start=True, stop=True,
                )
            cvec = rp.tile([128, DT], BF16)
            nc.vector.tensor_add(out=cvec[:], in0=cv_ps[:], in1=ceb_s[:])

            vcv_ps = rps.tile([128, DT], F32, tag="rp2")
            cawv_blk = rp.tile([128, DT, D], BF16, tag="rwblk")
            nc.sync.dma_start(
                out=cawv_blk[:], in_=cawv.rearrange("(a p) n -> p a n", p=128)
            )
            for m in range(DT):
                for k in range(DT):
                    nc.tensor.matmul(
                        vcv_ps[:, m : m + 1],
                        cawv_blk[:, k, m * 128 : (m + 1) * 128],
                        cvec[:, k : k + 1],
                        start=(k == 0), stop=(k == DT - 1),
                    )
            vcv = rp.tile([128, DT], BF16)
            nc.vector.tensor_add(out=vcv[:], in0=vcv_ps[:], in1=cabv_s[:])

            r_ps = rps.tile([128, DT], F32, tag="rp3")
            cawo_blk = rp.tile([128, DT, D], BF16, tag="rwblk2")
            nc.sync.dma_start(
                out=cawo_blk[:], in_=cawo.rearrange("(a p) n -> p a n", p=128)
            )
            for m in range(DT):
                for k in range(DT):
                    nc.tensor.matmul(
                        r_ps[:, m : m + 1],
                        cawo_blk[:, k, m * 128 : (m + 1) * 128],
                        vcv[:, k : k + 1],
                        start=(k == 0), stop=(k == DT - 1),
                    )
            rT = rp.tile([128, DT], F32)
            nc.vector.tensor_add(out=rT[:], in0=r_ps[:], in1=cabo_s[:])
            r_dram = dres.tile([D], F32)
            nc.sync.dma_start(
                out=r_dram.rearrange("(a p) -> p a", p=128), in_=rT[:]
            )
            nc.sync.dma_start(
                out=r_b[:], in_=r_dram[None, :].broadcast_to((128, D))
            )

        # -------- phases P+A share the big attention residents -------------
        with tc.tile_pool(name="pa", bufs=1) as pa:
            kT_s = pa.tile([128, DT, S], BF16)          # K.T (d-major), +bk
            vp_s = pa.tile([128, ST, H, HD + 1], BF16)  # V natural + ones col
            qT_s = pa.tile([128, DT, QS], BF16)         # Q.T (pre-scaled)
            maskT_s = pa.tile([128, ST, QS], BF16)
            nc.sync.dma_start(out=maskT_s[:], in_=maskT[:])
            xqT_s = pa.tile([128, DT, QS], BF16)
            nc.sync.dma_start(out=xqT_s[:], in_=xqT[:])

            # ---- phase P: K/V/Q projections -------------------------------
            with tc.tile_pool(name="pph", bufs=1) as pp, \
                 tc.tile_pool(name="pw", bufs=12) as pw, \
                 tc.tile_pool(name="pps", bufs=6, space="PSUM") as pps:
                xT_s = pp.tile([128, DT, S], BF16)
                nc.sync.dma_start(out=xT_s[:], in_=xT[:])
                bq_s = pp.tile([128, DT], F32)
                nc.sync.dma_start(out=bq_s[:], in_=bq_c[:])
                bk_s = pp.tile([128, DT], F32)
                nc.sync.dma_start(out=bk_s[:], in_=bk_c[:])
                bv_b = bcast_load(pp, bv_r, D, "bvb")

                nc.vector.memset(vp_s[:, :, :, HD : HD + 1], 1.0)

                # K: kT[:, m, ns] = wk[:, m].T @ xT  (+bk)
                wk_t = [pw.tile([128, D], BF16, tag="w", name=f"wk_t{_k}") for _k in range(DT)]
                for k in range(DT):
                    nc.sync.dma_start(
                        out=wk_t[k][:], in_=wk[k * 128 : (k + 1) * 128, :]
                    )
                for m in range(DT):
                    pss = [
                        pps.tile([128, 512], F32, tag="pj", name=f"kps{m}_{ns}")
                        for ns in range(S // 512)
                    ]
                    for k in range(DT):
                        for ns in range(S // 512):
                            nc.tensor.matmul(
                                pss[ns][:],
                                wk_t[k][:, m * 128 : (m + 1) * 128],
                                xT_s[:, k, ns * 512 : (ns + 1) * 512],
                                start=(k == 0), stop=(k == DT - 1),
                            )
                    for ns in range(S // 512):
                        nc.vector.tensor_scalar(
                            out=kT_s[:, m, ns * 512 : (ns + 1) * 512],
                            in0=pss[ns][:], scalar1=bk_s[:, m : m + 1],
                            scalar2=None, op0=ADD,
                        )

                # Q (own rows): qT[:, m, :] = wq[:, m].T @ xqT  (+bq)
                wq_t = [pw.tile([128, D], BF16, tag="w", name=f"wq_t{_k}") for _k in range(DT)]
                for k in range(DT):
                    nc.sync.dma_start(
                        out=wq_t[k][:], in_=wq[k * 128 : (k + 1) * 128, :]
                    )
                for m in range(DT):
                    ps = pps.tile([128, QS], F32, tag="pj")
                    for k in range(DT):
                        nc.tensor.matmul(
                            ps[:],
                            wq_t[k][:, m * 128 : (m + 1) * 128],
                            xqT_s[:, k, :],
                            start=(k == 0), stop=(k == DT - 1),
                        )
                    nc.vector.tensor_scalar(
                        out=qT_s[:, m, :], in0=ps[:],
                        scalar1=bq_s[:, m : m + 1], scalar2=None, op0=ADD,
                    )

                # V (natural): vp[:, st, heads, :64] = xT[:, :, st].T @ wv (+bv)
                wv_t = [pw.tile([128, D], BF16, tag="w", name=f"wv_t{_k}") for _k in range(DT)]
                for k in range(DT):
                    nc.sync.dma_start(
                        out=wv_t[k][:], in_=wv[k * 128 : (k + 1) * 128, :]
                    )
                for st in range(ST):
                    pss = [
                        pps.tile([128, 512], F32, tag="pj", name=f"vps{st}_{c}")
                        for c in range(D // 512)
                    ]
                    for k in range(DT):
                        for c in range(D // 512):
                            nc.tensor.matmul(
                                pss[c][:],
                                xT_s[:, k, st * 128 : (st + 1) * 128],
                                wv_t[k][:, c * 512 : (c + 1) * 512],
                                start=(k == 0), stop=(k == DT - 1),
                            )
                    for c in range(D // 512):
                        nc.vector.tensor_add(
                            out=vp_s[:, st, c * 8 : (c + 1) * 8, 0:HD],
                            in0=pss[c][:].rearrange("p (h e) -> p h e", e=HD),
                            in1=bv_b[:, c * 512 : (c + 1) * 512].rearrange(
                                "p (h e) -> p h e", e=HD
                            ),
                        )

            # ---- phase A: attention ---------------------------------------
            with tc.tile_pool(name="aph", bufs=4) as apl, \
                 tc.tile_pool(name="aps", bufs=3, space="PSUM") as aps, \
                 tc.tile_pool(name="avps", bufs=2, space="PSUM") as avps, \
                 tc.tile_pool(name="adr", bufs=3, space="DRAM") as adr:
                for h in range(H):
                    pb = (h % 2) * 64
                    dtile = h // 2
                    av = avps.tile([HD + 1, QS], F32, tag="av")
                    for jp in range(ST // 2):
                        qk = aps.tile([128, 2, 512], F32, tag="qk")
                        for hf in range(2):
                            j = jp * 2 + hf
                            nc.tensor.matmul(
                                qk[:, hf, :],
                                kT_s[pb : pb + 64, dtile, j * 128 : (j + 1) * 128],
                                qT_s[pb : pb + 64, dtile, :],
                                start=True, stop=True,
                            )
                        pt = apl.tile([128, 2, 512], BF16, tag="pt")
                        nc.scalar.activation(pt[:], qk[:], Exp)
                        nc.vector.tensor_mul(
                            out=pt[:], in0=pt[:],
                            in1=maskT_s[:, jp * 2 : jp * 2 + 2, :],
                        )
                        for hf in range(2):
                            j = jp * 2 + hf
                            nc.tensor.matmul(
                                av[:],
                                vp_s[:, j, h, :],
                                pt[:, hf, :],
                                start=(j == 0), stop=(j == ST - 1),
                            )
                    # divide by the ones-column sum, store transposed bf16
                    dr_row = apl.tile([1, QS], F32, tag="dr")
                    nc.vector.tensor_copy(out=dr_row[:], in_=av[HD : HD + 1, :])
                    dnb = adr.tile([1, QS], F32, tag="dnb")
                    nc.sync.dma_start(out=dnb[:], in_=dr_row[:])
                    rb = apl.tile([64, QS], F32, tag="rb")
                    nc.sync.dma_start(
                        out=rb[:], in_=dnb[0:1, :].broadcast_to((64, QS))
                    )
                    nc.vector.reciprocal(out=rb[:], in_=rb[:])
                    nc.vector.tensor_mul(
                        out=oT_s[pb : pb + 64, dtile, :], in0=av[0:HD, :],
                        in1=rb[:],
                    )

        # -------- phase O: out-proj, AddNorm, cross-attn row, AddNorm ------
        with tc.tile_pool(name="of", bufs=1) as of:
            h2_s = of.tile([128, QT, D], F32)
            h2T_s = of.tile([128, DT, QS], BF16)
            with tc.tile_pool(name="oph", bufs=1) as op, \
                 tc.tile_pool(name="ow", bufs=9) as ow, \
                 tc.tile_pool(name="ops", bufs=4, space="PSUM") as ops, \
                 tc.tile_pool(name="otps", bufs=2, space="PSUM") as otps, \
                 tc.tile_pool(name="oln", bufs=4) as oln:
                xr_s = op.tile([128, QT, D], F32)
                nc.sync.dma_start(
                    out=xr_s[:], in_=xrows.rearrange("(t p) d -> p t d", p=128)
                )
                bo_b = bcast_load(op, bo_r, D, "bob")
                g1_b = bcast_load(op, g1_r, D, "g1b")
                lb1_b = bcast_load(op, lb1_r, D, "lb1b")
                g2_b = bcast_load(op, g2_r, D, "g2b")
                lb2_b = bcast_load(op, lb2_r, D, "lb2b")

                wo_t = [ow.tile([128, D], BF16, tag="wo", name=f"wo_t{_k}") for _k in range(DT)]
                for k in range(DT):
                    nc.sync.dma_start(
                        out=wo_t[k][:], in_=wo[k * 128 : (k + 1) * 128, :]
                    )

                h_s = op.tile([128, QT, D], F32)
                for mq in range(QT):
                    nc.vector.tensor_add(
                        out=xr_s[:, mq, :], in0=xr_s[:, mq, :], in1=bo_b[:]
                    )
                for mq in range(QT):
                    pss = [
                        ops.tile([128, 512], F32, tag="op", name=f"ops{mq}_{ns}")
                        for ns in range(D // 512)
                    ]
                    for k in range(DT):
                        for ns in range(D // 512):
                            nc.tensor.matmul(
                                pss[ns][:],
                                oT_s[:, k, mq * 128 : (mq + 1) * 128],
                                wo_t[k][:, ns * 512 : (ns + 1) * 512],
                                start=(k == 0), stop=(k == DT - 1),
                            )
                    for ns in range(D // 512):
                        sl = slice(ns * 512, (ns + 1) * 512)
                        nc.vector.tensor_add(
                            out=h_s[:, mq, sl], in0=pss[ns][:], in1=xr_s[:, mq, sl]
                        )
                    layer_norm(oln, h_s[:, mq, :], h_s[:, mq, :], g1_b, lb1_b)
                    nc.vector.tensor_add(
                        out=h2_s[:, mq, :], in0=h_s[:, mq, :], in1=r_b[:]
                    )
                    layer_norm(oln, h2_s[:, mq, :], h2_s[:, mq, :], g2_b, lb2_b)
                    h2bf = oln.tile([128, D], BF16, tag="h2bf")
                    nc.vector.tensor_copy(out=h2bf[:], in_=h2_s[:, mq, :])
                    for t in range(DT):
                        tp = otps.tile([128, 128], BF16, tag="tp")
                        nc.tensor.transpose(
                            tp[:], h2bf[:, t * 128 : (t + 1) * 128], ident[:]
                        )
                        nc.vector.tensor_copy(
                            out=h2T_s[:, t, mq * 128 : (mq + 1) * 128], in_=tp[:]
                        )

            # -------- phase F: FFN + AddNorm -------------------------------
            with tc.tile_pool(name="fph", bufs=1) as fp, \
                 tc.tile_pool(name="fln", bufs=4) as fln:
                b1_s = fp.tile([128, FT], F32)
                nc.sync.dma_start(out=b1_s[:], in_=b1_c[:])
                b2_b = bcast_load(fp, b2_r, D, "b2b")
                g3_b = bcast_load(fp, g3_r, D, "g3b")
                lb3_b = bcast_load(fp, lb3_r, D, "lb3b")

                fT_s = fp.tile([128, FT, QS], BF16)
                # FF1: fT[:, mf, :] = relu(w1[:, mf].T @ h2T + b1)
                with tc.tile_pool(name="fw1", bufs=2) as fw1, \
                     tc.tile_pool(name="fps", bufs=3, space="PSUM") as fps:
                    for mfg in range(4):
                        w1_t = fw1.tile([128, DT, 1024], BF16, tag="w1")
                        nc.sync.dma_start(
                            out=w1_t[:],
                            in_=w1.rearrange("(a p) n -> p a n", p=128)[
                                :, :, mfg * 1024 : (mfg + 1) * 1024
                            ],
                        )
                        for mfl in range(8):
                            mf = mfg * 8 + mfl
                            ps = fps.tile([128, QS], F32, tag="f1")
                            for k in range(DT):
                                nc.tensor.matmul(
                                    ps[:],
                                    w1_t[:, k, mfl * 128 : (mfl + 1) * 128],
                                    h2T_s[:, k, :],
                                    start=(k == 0), stop=(k == DT - 1),
                                )
                            nc.vector.tensor_scalar(
                                out=ps[:], in0=ps[:],
                                scalar1=b1_s[:, mf : mf + 1], scalar2=None,
                                op0=ADD,
                            )
                            nc.scalar.activation(
                                out=fT_s[:, mf, :], in_=ps[:], func=Relu
                            )

                # FF2: ff rows = fT.T @ w2 (+b2) + h2, then LN3
                out_t = fp.tile([128, QT, D], F32)
                with tc.tile_pool(name="fw2", bufs=3) as fw2, \
                     tc.tile_pool(name="f2ps", bufs=8, space="PSUM") as f2ps:
                    ps2 = [
                        f2ps.tile([128, 512], F32, tag="f2", name=f"ps2_{_i}")
                        for _i in range(QT * (D // 512))
                    ]
                    for kf in range(FT):
                        w2_t = fw2.tile([128, D], BF16, tag="w2")
                        nc.sync.dma_start(
                            out=w2_t[:], in_=w2[kf * 128 : (kf + 1) * 128, :]
                        )
                        for mq in range(QT):
                            for ns in range(D // 512):
                                nc.tensor.matmul(
                                    ps2[mq * 2 + ns][:],
                                    fT_s[:, kf, mq * 128 : (mq + 1) * 128],
                                    w2_t[:, ns * 512 : (ns + 1) * 512],
                                    start=(kf == 0), stop=(kf == FT - 1),
                                )
                    for mq in range(QT):
                        nc.vector.tensor_add(
                            out=h2_s[:, mq, :], in0=h2_s[:, mq, :], in1=b2_b[:]
                        )
                    for mq in range(QT):
                        for ns in range(D // 512):
                            sl = slice(ns * 512, (ns + 1) * 512)
                            nc.vector.tensor_add(
                                out=out_t[:, mq, sl], in0=ps2[mq * 2 + ns][:],
                                in1=h2_s[:, mq, sl],
                            )
                        layer_norm(
                            fln, out_t[:, mq, :], out_t[:, mq, :], g3_b, lb3_b
                        )
                        nc.sync.dma_start(
                            out=out_d.rearrange("(t p) d -> p t d", p=128)[:, mq, :],
                            in_=out_t[:, mq, :],
                        )

    _split_sync_waits(nc)
    return nc


_NC_CACHE = None


def _get_program():
    global _NC_CACHE
    if _NC_CACHE is None:
        _NC_CACHE = _build_program()
    return _NC_CACHE


# ----------------------------------------------------------------------------
# host wrapper
# ----------------------------------------------------------------------------

def _col_interleave(v, nt):
    """[n] f32 -> [128, nt] where col j holds v[j*128:(j+1)*128]."""
    return np.ascontiguousarray(
        np.asarray(v, np.float32).reshape(nt, 128).T
    )


def kernel(**inputs):
    x = np.asarray(inputs["cur_input"], np.float32)          # [B, S, D]
    cls = np.asarray(inputs["classVector"], np.float32)      # [B, 1, 10]
    mask = np.asarray(inputs["attn_mask"])                   # [S, S] bool

    bf = lambda a: np.ascontiguousarray(np.asarray(a, np.float32)).astype(NP_BF16)
    f32 = lambda a: np.ascontiguousarray(np.asarray(a, np.float32))
    row = lambda v: f32(np.asarray(v, np.float32).reshape(1, -1))

    shared = dict(
        wq=bf(np.asarray(inputs["sa_wq"], np.float32) * SCALE),
        wk=bf(inputs["sa_wk"]),
        wv=bf(inputs["sa_wv"]),
        wo=bf(inputs["sa_wo"]),
        w1=bf(inputs["ff_w1"]),
        w2=bf(inputs["ff_w2"]),
        cew=bf(inputs["ce_w"]),
        cawv=bf(inputs["ca_wv"]),
        cawo=bf(inputs["ca_wo"]),
        bq_c=_col_interleave(np.asarray(inputs["sa_bq"], np.float32) * SCALE, DT),
        bk_c=_col_interleave(inputs["sa_bk"], DT),
        b1_c=_col_interleave(inputs["ff_b1"], FT),
        ceb_c=_col_interleave(inputs["ce_b"], DT),
        cabv_c=_col_interleave(inputs["ca_bv"], DT),
        cabo_c=_col_interleave(inputs["ca_bo"], DT),
        bv_r=row(inputs["sa_bv"]),
        bo_r=row(inputs["sa_bo"]),
        b2_r=row(inputs["ff_b2"]),
        g1_r=row(inputs["ln1_g"]),
        lb1_r=row(inputs["ln1_b"]),
        g2_r=row(inputs["ln2_g"]),
        lb2_r=row(inputs["ln2_b"]),
        g3_r=row(inputs["ln3_g"]),
        lb3_r=row(inputs["ln3_b"]),
    )

    mT = mask.T.astype(np.float32)  # [S key, S query]
    in_maps = []
    for c in range(NCORES):
        b, q0 = c // (NCORES // B), (c % (NCORES // B)) * QS
        xTb = x[b].T.reshape(DT, 128, S).transpose(1, 0, 2)       # [128, DT, S]
        mTc = mT[:, q0 : q0 + QS].reshape(ST, 128, QS).transpose(1, 0, 2)
        in_maps.append(
            dict(
                shared,
                xT=bf(xTb),
                xqT=bf(xTb[:, :, q0 : q0 + QS]),
                xrows=f32(x[b, q0 : q0 + QS, :]),
                maskT=bf(mTc),
                cvT=bf(cls[b, 0].reshape(10, 1)),
            )
        )

    res = run_bass_kernel_spmd(_get_program(), in_maps, list(range(NCORES)))
    out = np.empty((B, S, D), np.float32)
    for c in range(NCORES):
        b, q0 = c // (NCORES // B), (c % (NCORES // B)) * QS
        out[b, q0 : q0 + QS] = res.results[c]["out"]
    return out


# revision 5
# speedup vs baseline: 1.1436x; 1.0883x over previous
"""Trainium2 Bass kernel for nn_Block_86672440033530 (sparse_attention).

Transformer block: masked self-attention + AddNorm, class-vector cross-attn
(collapses to a broadcast row since Sk=1) + AddNorm, FFN + AddNorm.

Sharding: 8 cores = 2 batches x 4 query-blocks of 512 rows. Each core
computes full K/V projections for its batch (replicated within the 4-core
batch group), attention for its 512 query rows over all 2048 keys and all 16
heads, then output-projection / LayerNorms / FFN for its rows only. No
cross-core communication; host gathers the 8 row-blocks.

All matmuls run in bf16 (fp32 PSUM accumulation); norms/softmax plumbing in
fp32. The softmax is computed as exp(S)*mask with the denominator taken from
a ones-column appended to V (fused into the AV matmul) and divided out during
PSUM eviction via a DRAM-bounce partition broadcast.
"""
import contextlib
import ctypes
import sys
import types

import numpy as np

if "/opt/trn_rl_repo" not in sys.path:
    sys.path.insert(0, "/opt/trn_rl_repo")

import ml_dtypes  # noqa: E402
import concourse.bass as bass  # noqa: E402
import concourse.mybir as mybir  # noqa: E402
import concourse.tile as tile  # noqa: E402
from concourse.bass_utils import run_bass_kernel_spmd  # noqa: E402
from concourse.masks import make_identity  # noqa: E402

BF16 = mybir.dt.bfloat16
F32 = mybir.dt.float32
NP_BF16 = ml_dtypes.bfloat16

B, S, D, H, DFF = 2, 2048, 1024, 16, 4096
HD = D // H                      # 64
SCALE = float(1.0 / np.sqrt(np.float32(HD)))
NCORES = 8
QS = S // (NCORES // B)          # 512 query rows per core
QT = QS // 128                   # 4 query tiles per core
DT = D // 128                    # 8 d-blocks
ST = S // 128                    # 16 key tiles
FT = DFF // 128                  # 32 dff tiles
EPS = 1e-5


def _install_ntff_shim():
    """The axon image lacks antenv.axon_hooks; register the NTFF profile hook
    via ctypes so run_bass_kernel_spmd(trace=True) works. Harmless if unused."""
    try:
        import antenv
    except ImportError:
        return
    if "antenv.axon_hooks" in sys.modules:
        return

    def _make_hook(so_path):
        try:
            lib = ctypes.CDLL(so_path)
        except OSError:
            return None
        if not hasattr(lib, "axon_start_nrt_profile"):
            return None
        lib.axon_start_nrt_profile.argtypes = [
            ctypes.POINTER(ctypes.c_int64),
            ctypes.c_size_t,
        ]
        lib.axon_start_nrt_profile.restype = ctypes.c_int64
        lib.axon_stop_nrt_profile.argtypes = [ctypes.c_char_p]
        lib.axon_stop_nrt_profile.restype = ctypes.c_int64

        @contextlib.contextmanager
        def _hook(output_dir, device_ids):
            import jax

            jax.devices()
            if device_ids:
                ids = (ctypes.c_int64 * len(device_ids))(*device_ids)
                rc = lib.axon_start_nrt_profile(ids, len(device_ids))
            else:
                rc = lib.axon_start_nrt_profile(None, 0)
            if rc != 0:
                raise RuntimeError(f"axon_start_nrt_profile rc={rc}")
            try:
                yield
            finally:
                n = lib.axon_stop_nrt_profile(str(output_dir).encode())
                print(f"profile: {n} file(s) -> {output_dir}", file=sys.stderr)

        return _hook

    m = types.ModuleType("antenv.axon_hooks")
    m._hook = _make_hook("/opt/axon/libaxon_pjrt.so")
    m.set_axon_ntff_profile_hook = lambda h: setattr(m, "_hook", h)
    m.get_axon_ntff_profile_hook = lambda: m._hook
    sys.modules["antenv.axon_hooks"] = m
    import antenv

    antenv.axon_hooks = m


_install_ntff_shim()


def _split_sync_waits(nc, limit=1):
    """This walrus build accepts at most one sync-wait command per
    instruction; move excess waits onto same-engine NoOps placed before."""
    for func in nc.m.functions:
        for bb in func.blocks:
            out = []
            for ins in bb.instructions:
                si = getattr(ins, "sync_info", None)
                waits = list(si.on_wait) if (si is not None and si.on_wait) else []
                if len(waits) > limit:
                    keep, move = waits[:limit], waits[limit:]
                    for i in range(0, len(move), limit):
                        out.append(
                            mybir.InstNoOp(
                                name=f"{ins.name}-wsplit{i}",
                                sync_info=mybir.SyncInfo(
                                    on_wait=move[i : i + limit], on_update=[]
                                ),
                                bass_nofuse=True,
                                engine=ins.engine,
                            )
                        )
                    si.on_wait = keep
                out.append(ins)
            bb.instructions[:] = out


# ----------------------------------------------------------------------------
# device program (SPMD; identical on all 8 cores, per-core data differs)
# ----------------------------------------------------------------------------

def _build_program():
    nc = bass.Bass()

    def din(name, shape, dt):
        return nc.dram_tensor(name, list(shape), dt, kind="ExternalInput")

    # per-core tensors
    xT = din("xT", [128, DT, S], BF16)          # x[b].T  (d-major)
    xqT = din("xqT", [128, DT, QS], BF16)       # own q rows of xT
    xrows = din("xrows", [QS, D], F32)          # own q rows, natural (residual)
    maskT = din("maskT", [128, ST, QS], BF16)   # mask.T own q cols, tile-major
    cvT = din("cvT", [10, 1], BF16)             # classVector[b].T
    # weights (bf16, shared; wq/bq pre-scaled by 1/sqrt(hd))
    wq = din("wq", [D, D], BF16)
    wk = din("wk", [D, D], BF16)
    wv = din("wv", [D, D], BF16)
    wo = din("wo", [D, D], BF16)
    w1 = din("w1", [D, DFF], BF16)
    w2 = din("w2", [DFF, D], BF16)
    cew = din("cew", [10, D], BF16)
    cawv = din("cawv", [D, D], BF16)
    cawo = din("cawo", [D, D], BF16)
    # f32 bias/ln vectors: column-interleaved [128, n] or rows [1, n]
    bq_c = din("bq_c", [128, DT], F32)
    bk_c = din("bk_c", [128, DT], F32)
    b1_c = din("b1_c", [128, FT], F32)
    ceb_c = din("ceb_c", [128, DT], F32)
    cabv_c = din("cabv_c", [128, DT], F32)
    cabo_c = din("cabo_c", [128, DT], F32)
    bv_r = din("bv_r", [1, D], F32)
    bo_r = din("bo_r", [1, D], F32)
    b2_r = din("b2_r", [1, D], F32)
    g1_r = din("g1_r", [1, D], F32)
    lb1_r = din("lb1_r", [1, D], F32)
    g2_r = din("g2_r", [1, D], F32)
    lb2_r = din("lb2_r", [1, D], F32)
    g3_r = din("g3_r", [1, D], F32)
    lb3_r = din("lb3_r", [1, D], F32)

    out_d = nc.dram_tensor("out", [QS, D], F32, kind="ExternalOutput")

    Exp = mybir.ActivationFunctionType.Exp
    Relu = mybir.ActivationFunctionType.Relu
    Sqrt = mybir.ActivationFunctionType.Sqrt
    ADD = mybir.AluOpType.add
    SUB = mybir.AluOpType.subtract
    MUL = mybir.AluOpType.mult

    with tile.TileContext(nc) as tc, contextlib.ExitStack() as ctx:
        # -------- whole-kernel residents (small) ---------------------------
        res = ctx.enter_context(tc.tile_pool(name="res", bufs=1))
        dres = ctx.enter_context(tc.tile_pool(name="dres", bufs=1, space="DRAM"))

        ident = res.tile([128, 128], BF16)
        make_identity(nc, ident)
        eps_t = res.tile([128, 1], F32)
        nc.vector.memset(eps_t[:], EPS)
        oT_s = res.tile([128, DT, QS], BF16)     # attention output (transposed)
        r_b = res.tile([128, D], F32)            # cross-attn row, broadcast

        def bcast_load(pool, src_row, n, tag):
            t = pool.tile([128, n], F32, tag=tag)
            nc.sync.dma_start(out=t[:], in_=src_row[0:1, :].broadcast_to((128, n)))
            return t

        def layer_norm(pool, dst, src, g_b, lb_b):
            """dst = LN_freedim(src) * g + b for [128, D] f32 views."""
            stats = pool.tile([128, 2, 6], F32, tag="lnst")
            mv = pool.tile([128, 2], F32, tag="lnmv")
            for sg in range(2):
                nc.vector.bn_stats(
                    out=stats[:, sg, :], in_=src[:, sg * 512 : (sg + 1) * 512]
                )
            nc.vector.bn_aggr(out=mv[:], in_=stats[:])
            rstd = pool.tile([128, 1], F32, tag="lnrs")
            nc.scalar.activation(
                out=rstd[:], in_=mv[:, 1:2], func=Sqrt, bias=eps_t[:]
            )
            nc.vector.reciprocal(out=rstd[:], in_=rstd[:])
            nc.vector.tensor_scalar(
                out=dst[:], in0=src[:], scalar1=mv[:, 0:1], scalar2=rstd[:],
                op0=SUB, op1=MUL,
            )
            nc.vector.tensor_mul(out=dst[:], in0=dst[:], in1=g_b[:])
            nc.vector.tensor_add(out=dst[:], in0=dst[:], in1=lb_b[:])

        # -------- phase R: cross-attn row r --------------------------------
        # r = ((classVector @ ce_w + ce_b) @ ca_wv + ca_bv) @ ca_wo + ca_bo,
        # in column form rT [128, DT]; softmax over a single key == 1 so the
        # q/k cross-attn projections drop out entirely.
        with tc.tile_pool(name="rph", bufs=1) as rp, \
             tc.tile_pool(name="rps", bufs=2, space="PSUM") as rps:
            cv_bf = rp.tile([10, 1], BF16)
            nc.sync.dma_start(out=cv_bf[:], in_=cvT[:])
            cew_s = rp.tile([10, D], BF16)
            nc.sync.dma_start(out=cew_s[:], in_=cew[:])
            ceb_s = rp.tile([128, DT], F32)
            nc.sync.dma_start(out=ceb_s[:], in_=ceb_c[:])
            cabv_s = rp.tile([128, DT], F32)
            nc.sync.dma_start(out=cabv_s[:], in_=cabv_c[:])
            cabo_s = rp.tile([128, DT], F32)
            nc.sync.dma_start(out=cabo_s[:], in_=cabo_c[:])

            cv_ps = rps.tile([128, DT], F32, tag="rp1")
            for m in range(DT):
                nc.tensor.matmul(
                    cv_ps[:, m : m + 1], cew_s[:, m * 128 : (m + 1) * 128],
                    cv_bf[:], start=True, stop=True,
                )
            cvec = rp.tile([128, DT], BF16)
            nc.vector.tensor_add(out=cvec[:], in0=cv_ps[:], in1=ceb_s[:])

            vcv_ps = rps.tile([128, DT], F32, tag="rp2")
            cawv_blk = rp.tile([128, DT, D], BF16, tag="rwblk")
            nc.sync.dma_start(
                out=cawv_blk[:], in_=cawv.rearrange("(a p) n -> p a n", p=128)
            )
            for m in range(DT):
                for k in range(DT):
                    nc.tensor.matmul(
                        vcv_ps[:, m : m + 1],
                        cawv_blk[:, k, m * 128 : (m + 1) * 128],
                        cvec[:, k : k + 1],
                        start=(k == 0), stop=(k == DT - 1),
                    )
            vcv = rp.tile([128, DT], BF16)
            nc.vector.tensor_add(out=vcv[:], in0=vcv_ps[:], in1=cabv_s[:])

            r_ps = rps.tile([128, DT], F32, tag="rp3")
            cawo_blk = rp.tile([128, DT, D], BF16, tag="rwblk2")
            nc.sync.dma_start(
                out=cawo_blk[:], in_=cawo.rearrange("(a p) n -> p a n", p=128)
            )
            for m in range(DT):
                for k in range(DT):
                    nc.tensor.matmul(
                        r_ps[:, m : m + 1],
                        cawo_blk[:, k, m * 128 : (m + 1) * 128],
                        vcv[:, k : k + 1],
                        start=(k == 0), stop=(k == DT - 1),
                    )
            rT = rp.tile([128, DT], F32)
            nc.vector.tensor_add(out=rT[:], in0=r_ps[:], in1=cabo_s[:])
            r_dram = dres.tile([D], F32)
            nc.sync.dma_start(
                out=r_dram.rearrange("(a p) -> p a", p=128), in_=rT[:]
            )
            nc.sync.dma_start(
                out=r_b[:], in_=r_dram[None, :].broadcast_to((128, D))
            )

        # -------- phases P+A share the big attention residents -------------
        with tc.tile_pool(name="pa", bufs=1) as pa:
            kT_s = pa.tile([128, DT, S], BF16)          # K.T (d-major), +bk
            vp_s = pa.tile([128, ST, H, HD + 1], BF16)  # V natural + ones col
            qT_s = pa.tile([128, H, QS], BF16)          # Q.T zero-padded to K=128
            maskT_s = pa.tile([128, ST, QS], BF16)
            nc.sync.dma_start(out=maskT_s[:], in_=maskT[:])
            xqT_s = pa.tile([128, DT, QS], BF16)
            nc.sync.dma_start(out=xqT_s[:], in_=xqT[:])

            # ---- phase P: K/V/Q projections -------------------------------
            with tc.tile_pool(name="pph", bufs=1) as pp, \
                 tc.tile_pool(name="pw", bufs=12) as pw, \
                 tc.tile_pool(name="pps", bufs=6, space="PSUM") as pps:
                xT_s = pp.tile([128, DT, S], BF16)
                nc.sync.dma_start(out=xT_s[:], in_=xT[:])
                bq_s = pp.tile([128, DT], F32)
                nc.sync.dma_start(out=bq_s[:], in_=bq_c[:])
                bk_s = pp.tile([128, DT], F32)
                nc.sync.dma_start(out=bk_s[:], in_=bk_c[:])
                bv_b = bcast_load(pp, bv_r, D, "bvb")

                nc.vector.memset(vp_s[:, :, :, HD : HD + 1], 1.0)

                # K: kT[:, m, ns] = wk[:, m].T @ xT  (+bk)
                wk_t = [pw.tile([128, D], BF16, tag="w", name=f"wk_t{_k}") for _k in range(DT)]
                for k in range(DT):
                    nc.sync.dma_start(
                        out=wk_t[k][:], in_=wk[k * 128 : (k + 1) * 128, :]
                    )
                for m in range(DT):
                    pss = [
                        pps.tile([128, 512], F32, tag="pj", name=f"kps{m}_{ns}")
                        for ns in range(S // 512)
                    ]
                    for k in range(DT):
                        for ns in range(S // 512):
                            nc.tensor.matmul(
                                pss[ns][:],
                                wk_t[k][:, m * 128 : (m + 1) * 128],
                                xT_s[:, k, ns * 512 : (ns + 1) * 512],
                                start=(k == 0), stop=(k == DT - 1),
                            )
                    for ns in range(S // 512):
                        nc.vector.tensor_scalar(
                            out=kT_s[:, m, ns * 512 : (ns + 1) * 512],
                            in0=pss[ns][:], scalar1=bk_s[:, m : m + 1],
                            scalar2=None, op0=ADD,
                        )

                # Q (own rows), zero-padded per head so QK runs at K=128:
                # even head data in partitions 0-63, odd in 64-127.
                nc.vector.memset(qT_s[64:128, 0:H:2, :], 0.0)
                nc.vector.memset(qT_s[0:64, 1:H:2, :], 0.0)
                wq_t = [pw.tile([128, D], BF16, tag="w", name=f"wq_t{_k}") for _k in range(DT)]
                for k in range(DT):
                    nc.sync.dma_start(
                        out=wq_t[k][:], in_=wq[k * 128 : (k + 1) * 128, :]
                    )
                for m in range(DT):
                    ps = pps.tile([128, QS], F32, tag="pj")
                    for k in range(DT):
                        nc.tensor.matmul(
                            ps[:],
                            wq_t[k][:, m * 128 : (m + 1) * 128],
                            xqT_s[:, k, :],
                            start=(k == 0), stop=(k == DT - 1),
                        )
                    nc.vector.tensor_scalar(
                        out=qT_s[0:64, 2 * m, :], in0=ps[0:64, :],
                        scalar1=bq_s[0:64, m : m + 1], scalar2=None, op0=ADD,
                    )
                    nc.vector.tensor_scalar(
                        out=qT_s[64:128, 2 * m + 1, :], in0=ps[64:128, :],
                        scalar1=bq_s[64:128, m : m + 1], scalar2=None, op0=ADD,
                    )

                # V (natural): vp[:, st, heads, :64] = xT[:, :, st].T @ wv (+bv)
                wv_t = [pw.tile([128, D], BF16, tag="w", name=f"wv_t{_k}") for _k in range(DT)]
                for k in range(DT):
                    nc.sync.dma_start(
                        out=wv_t[k][:], in_=wv[k * 128 : (k + 1) * 128, :]
                    )
                for st in range(ST):
                    pss = [
                        pps.tile([128, 512], F32, tag="pj", name=f"vps{st}_{c}")
                        for c in range(D // 512)
                    ]
                    for k in range(DT):
                        for c in range(D // 512):
                            nc.tensor.matmul(
                                pss[c][:],
                                xT_s[:, k, st * 128 : (st + 1) * 128],
                                wv_t[k][:, c * 512 : (c + 1) * 512],
                                start=(k == 0), stop=(k == DT - 1),
                            )
                    for c in range(D // 512):
                        nc.vector.tensor_add(
                            out=vp_s[:, st, c * 8 : (c + 1) * 8, 0:HD],
                            in0=pss[c][:].rearrange("p (h e) -> p h e", e=HD),
                            in1=bv_b[:, c * 512 : (c + 1) * 512].rearrange(
                                "p (h e) -> p h e", e=HD
                            ),
                        )

            # ---- phase A: attention ---------------------------------------
            with tc.tile_pool(name="aph", bufs=4) as apl, \
                 tc.tile_pool(name="aps", bufs=3, space="PSUM") as aps, \
                 tc.tile_pool(name="avps", bufs=2, space="PSUM") as avps, \
                 tc.tile_pool(name="adr", bufs=3, space="DRAM") as adr:
                for h in range(H):
                    pb = (h % 2) * 64
                    dtile = h // 2
                    av = avps.tile([HD + 1, QS], F32, tag="av")
                    for jp in range(ST // 2):
                        qk = aps.tile([128, 2, 512], F32, tag="qk")
                        for hf in range(2):
                            j = jp * 2 + hf
                            nc.tensor.matmul(
                                qk[:, hf, :],
                                kT_s[:, dtile, j * 128 : (j + 1) * 128],
                                qT_s[:, h, :],
                                start=True, stop=True,
                            )
                        pt = apl.tile([128, 2, 512], BF16, tag="pt")
                        nc.scalar.activation(pt[:], qk[:], Exp)
                        nc.vector.tensor_mul(
                            out=pt[:], in0=pt[:],
                            in1=maskT_s[:, jp * 2 : jp * 2 + 2, :],
                        )
                        for hf in range(2):
                            j = jp * 2 + hf
                            nc.tensor.matmul(
                                av[:],
                                vp_s[:, j, h, :],
                                pt[:, hf, :],
                                start=(j == 0), stop=(j == ST - 1),
                            )
                    # divide by the ones-column sum, store transposed bf16
                    dr_row = apl.tile([1, QS], F32, tag="dr")
                    nc.vector.tensor_copy(out=dr_row[:], in_=av[HD : HD + 1, :])
                    dnb = adr.tile([1, QS], F32, tag="dnb")
                    nc.sync.dma_start(out=dnb[:], in_=dr_row[:])
                    rb = apl.tile([64, QS], F32, tag="rb")
                    nc.sync.dma_start(
                        out=rb[:], in_=dnb[0:1, :].broadcast_to((64, QS))
                    )
                    nc.vector.reciprocal(out=rb[:], in_=rb[:])
                    nc.vector.tensor_mul(
                        out=oT_s[pb : pb + 64, dtile, :], in0=av[0:HD, :],
                        in1=rb[:],
                    )

        # -------- phase O: out-proj, AddNorm, cross-attn row, AddNorm ------
        with tc.tile_pool(name="of", bufs=1) as of:
            h2_s = of.tile([128, QT, D], F32)
            h2T_s = of.tile([128, DT, QS], BF16)
            with tc.tile_pool(name="oph", bufs=1) as op, \
                 tc.tile_pool(name="ow", bufs=9) as ow, \
                 tc.tile_pool(name="ops", bufs=4, space="PSUM") as ops, \
                 tc.tile_pool(name="otps", bufs=2, space="PSUM") as otps, \
                 tc.tile_pool(name="oln", bufs=4) as oln:
                xr_s = op.tile([128, QT, D], F32)
                nc.sync.dma_start(
                    out=xr_s[:], in_=xrows.rearrange("(t p) d -> p t d", p=128)
                )
                bo_b = bcast_load(op, bo_r, D, "bob")
                g1_b = bcast_load(op, g1_r, D, "g1b")
                lb1_b = bcast_load(op, lb1_r, D, "lb1b")
                g2_b = bcast_load(op, g2_r, D, "g2b")
                lb2_b = bcast_load(op, lb2_r, D, "lb2b")

                wo_t = [ow.tile([128, D], BF16, tag="wo", name=f"wo_t{_k}") for _k in range(DT)]
                for k in range(DT):
                    nc.sync.dma_start(
                        out=wo_t[k][:], in_=wo[k * 128 : (k + 1) * 128, :]
                    )

                h_s = op.tile([128, QT, D], F32)
                for mq in range(QT):
                    nc.vector.tensor_add(
                        out=xr_s[:, mq, :], in0=xr_s[:, mq, :], in1=bo_b[:]
                    )
                for mq in range(QT):
                    pss = [
                        ops.tile([128, 512], F32, tag="op", name=f"ops{mq}_{ns}")
                        for ns in range(D // 512)
                    ]
                    for k in range(DT):
                        for ns in range(D // 512):
                            nc.tensor.matmul(
                                pss[ns][:],
                                oT_s[:, k, mq * 128 : (mq + 1) * 128],
                                wo_t[k][:, ns * 512 : (ns + 1) * 512],
                                start=(k == 0), stop=(k == DT - 1),
                            )
                    for ns in range(D // 512):
                        sl = slice(ns * 512, (ns + 1) * 512)
                        nc.vector.tensor_add(
                            out=h_s[:, mq, sl], in0=pss[ns][:], in1=xr_s[:, mq, sl]
                        )
                    layer_norm(oln, h_s[:, mq, :], h_s[:, mq, :], g1_b, lb1_b)
                    nc.vector.tensor_add(
                        out=h2_s[:, mq, :], in0=h_s[:, mq, :], in1=r_b[:]
                    )
                    layer_norm(oln, h2_s[:, mq, :], h2_s[:, mq, :], g2_b, lb2_b)
                    h2bf = oln.tile([128, D], BF16, tag="h2bf")
                    nc.vector.tensor_copy(out=h2bf[:], in_=h2_s[:, mq, :])
                    for t in range(DT):
                        tp = otps.tile([128, 128], BF16, tag="tp")
                        nc.tensor.transpose(
                            tp[:], h2bf[:, t * 128 : (t + 1) * 128], ident[:]
                        )
                        nc.vector.tensor_copy(
                            out=h2T_s[:, t, mq * 128 : (mq + 1) * 128], in_=tp[:]
                        )

            # -------- phase F: FFN + AddNorm -------------------------------
            with tc.tile_pool(name="fph", bufs=1) as fp, \
                 tc.tile_pool(name="fln", bufs=4) as fln:
                b1_s = fp.tile([128, FT], F32)
                nc.sync.dma_start(out=b1_s[:], in_=b1_c[:])
                b2_b = bcast_load(fp, b2_r, D, "b2b")
                g3_b = bcast_load(fp, g3_r, D, "g3b")
                lb3_b = bcast_load(fp, lb3_r, D, "lb3b")

                fT_s = fp.tile([128, FT, QS], BF16)
                # FF1: fT[:, mf, :] = relu(w1[:, mf].T @ h2T + b1)
                with tc.tile_pool(name="fw1", bufs=2) as fw1, \
                     tc.tile_pool(name="fps", bufs=3, space="PSUM") as fps:
                    for mfg in range(4):
                        w1_t = fw1.tile([128, DT, 1024], BF16, tag="w1")
                        nc.sync.dma_start(
                            out=w1_t[:],
                            in_=w1.rearrange("(a p) n -> p a n", p=128)[
                                :, :, mfg * 1024 : (mfg + 1) * 1024
                            ],
                        )
                        for mfl in range(8):
                            mf = mfg * 8 + mfl
                            ps = fps.tile([128, QS], F32, tag="f1")
                            for k in range(DT):
                                nc.tensor.matmul(
                                    ps[:],
                                    w1_t[:, k, mfl * 128 : (mfl + 1) * 128],
                                    h2T_s[:, k, :],
                                    start=(k == 0), stop=(k == DT - 1),
                                )
                            nc.vector.tensor_scalar(
                                out=ps[:], in0=ps[:],
                                scalar1=b1_s[:, mf : mf + 1], scalar2=None,
                                op0=ADD,
                            )
                            nc.scalar.activation(
                                out=fT_s[:, mf, :], in_=ps[:], func=Relu
                            )

                # FF2: ff rows = fT.T @ w2 (+b2) + h2, then LN3
                out_t = fp.tile([128, QT, D], F32)
                with tc.tile_pool(name="fw2", bufs=3) as fw2, \
                     tc.tile_pool(name="f2ps", bufs=8, space="PSUM") as f2ps:
                    ps2 = [
                        f2ps.tile([128, 512], F32, tag="f2", name=f"ps2_{_i}")
                        for _i in range(QT * (D // 512))
                    ]
                    for kf in range(FT):
                        w2_t = fw2.tile([128, D], BF16, tag="w2")
                        nc.sync.dma_start(
                            out=w2_t[:], in_=w2[kf * 128 : (kf + 1) * 128, :]
                        )
                        for mq in range(QT):
                            for ns in range(D // 512):
                                nc.tensor.matmul(
                                    ps2[mq * 2 + ns][:],
                                    fT_s[:, kf, mq * 128 : (mq + 1) * 128],
                                    w2_t[:, ns * 512 : (ns + 1) * 512],
                                    start=(kf == 0), stop=(kf == FT - 1),
                                )
                    for mq in range(QT):
                        nc.vector.tensor_add(
                            out=h2_s[:, mq, :], in0=h2_s[:, mq, :], in1=b2_b[:]
                        )
                    for mq in range(QT):
                        for ns in range(D // 512):
                            sl = slice(ns * 512, (ns + 1) * 512)
                            nc.vector.tensor_add(
                                out=out_t[:, mq, sl], in0=ps2[mq * 2 + ns][:],
                                in1=h2_s[:, mq, sl],
                            )
                        layer_norm(
                            fln, out_t[:, mq, :], out_t[:, mq, :], g3_b, lb3_b
                        )
                        nc.sync.dma_start(
                            out=out_d.rearrange("(t p) d -> p t d", p=128)[:, mq, :],
                            in_=out_t[:, mq, :],
                        )

    _split_sync_waits(nc)
    return nc


_NC_CACHE = None


def _get_program():
    global _NC_CACHE
    if _NC_CACHE is None:
        _NC_CACHE = _build_program()
    return _NC_CACHE


# ----------------------------------------------------------------------------
# host wrapper
# ----------------------------------------------------------------------------

def _col_interleave(v, nt):
    """[n] f32 -> [128, nt] where col j holds v[j*128:(j+1)*128]."""
    return np.ascontiguousarray(
        np.asarray(v, np.float32).reshape(nt, 128).T
    )


def kernel(**inputs):
    x = np.asarray(inputs["cur_input"], np.float32)          # [B, S, D]
    cls = np.asarray(inputs["classVector"], np.float32)      # [B, 1, 10]
    mask = np.asarray(inputs["attn_mask"])                   # [S, S] bool

    bf = lambda a: np.ascontiguousarray(np.asarray(a, np.float32)).astype(NP_BF16)
    f32 = lambda a: np.ascontiguousarray(np.asarray(a, np.float32))
    row = lambda v: f32(np.asarray(v, np.float32).reshape(1, -1))

    shared = dict(
        wq=bf(np.asarray(inputs["sa_wq"], np.float32) * SCALE),
        wk=bf(inputs["sa_wk"]),
        wv=bf(inputs["sa_wv"]),
        wo=bf(inputs["sa_wo"]),
        w1=bf(inputs["ff_w1"]),
        w2=bf(inputs["ff_w2"]),
        cew=bf(inputs["ce_w"]),
        cawv=bf(inputs["ca_wv"]),
        cawo=bf(inputs["ca_wo"]),
        bq_c=_col_interleave(np.asarray(inputs["sa_bq"], np.float32) * SCALE, DT),
        bk_c=_col_interleave(inputs["sa_bk"], DT),
        b1_c=_col_interleave(inputs["ff_b1"], FT),
        ceb_c=_col_interleave(inputs["ce_b"], DT),
        cabv_c=_col_interleave(inputs["ca_bv"], DT),
        cabo_c=_col_interleave(inputs["ca_bo"], DT),
        bv_r=row(inputs["sa_bv"]),
        bo_r=row(inputs["sa_bo"]),
        b2_r=row(inputs["ff_b2"]),
        g1_r=row(inputs["ln1_g"]),
        lb1_r=row(inputs["ln1_b"]),
        g2_r=row(inputs["ln2_g"]),
        lb2_r=row(inputs["ln2_b"]),
        g3_r=row(inputs["ln3_g"]),
        lb3_r=row(inputs["ln3_b"]),
    )

    mT = mask.T.astype(np.float32)  # [S key, S query]
    in_maps = []
    for c in range(NCORES):
        b, q0 = c // (NCORES // B), (c % (NCORES // B)) * QS
        xTb = x[b].T.reshape(DT, 128, S).transpose(1, 0, 2)       # [128, DT, S]
        mTc = mT[:, q0 : q0 + QS].reshape(ST, 128, QS).transpose(1, 0, 2)
        in_maps.append(
            dict(
                shared,
                xT=bf(xTb),
                xqT=bf(xTb[:, :, q0 : q0 + QS]),
                xrows=f32(x[b, q0 : q0 + QS, :]),
                maskT=bf(mTc),
                cvT=bf(cls[b, 0].reshape(10, 1)),
            )
        )

    res = run_bass_kernel_spmd(_get_program(), in_maps, list(range(NCORES)))
    out = np.empty((B, S, D), np.float32)
    for c in range(NCORES):
        b, q0 = c // (NCORES // B), (c % (NCORES // B)) * QS
        out[b, q0 : q0 + QS] = res.results[c]["out"]
    return out
